# revision 4
# baseline (speedup 1.0000x reference)
"""ActorCriticGNN MAPPO forward on 8 Trainium2 NeuronCores (Bass/Tile).

Strategy
--------
GCNConv(x, W, b) = A_hat @ (x W) + b with A_hat = D^-1/2 (A+I) D^-1/2, and
A_hat @ (x W) = (A_hat @ x) W, so each conv is: sparse propagation, then a
dense 128x128 matmul. Actor and critic layer-1 share the propagation of x
(one pass), and layer-2 actor/critic propagations fuse into one 256-wide
pass over the concatenated table [a1|c1]. Only 2 sparse passes total.

Sharding: nodes (and their in-edges) are range-partitioned across the 8
cores. The layer-1 table is the replicated input x; the layer-2 table is
exchanged with one AllGather. Per-graph pooled sums use an AllReduce.

Per core, each propagation processes its ~200K in-edges in "supers" of 256
destination nodes. Edge source rows are fetched with dma_gather (int16
indices -> 4 chunk sub-tables of <=32768 rows). The segment sum runs on the
tensor engine: for each 128-slot K-tile, a one-hot matrix
S[slot, dst] = (iota == dstlocal[slot]) * norm[slot] is built in a single
DVE tensor_scalar op, and psum[feat, dst] += gathered_tile.T @ S
accumulates the normalized sums. norm = dinv[src]*dinv[dst] (dinv[dst]^2 for
the self-loop slots) carries the full GCN normalization, so tables are
gathered raw. All per-edge index math (sorting, padding, norm values) is
host-side preprocessing; all FLOPs on features run on device.
"""
from contextlib import ExitStack

import numpy as np

# ---------------------------------------------------------------- config
F, H, A, G, NC = 128, 128, 8, 64, 8
SUP = 256            # destination nodes per super-group
CHUNK = 32768        # gather sub-table rows (int16 index reach)
CALL_TILES = 6       # <=768 indices per dma_gather call (HW limit ~1K)
NQ = 4               # SWDGE queues for gather descriptor generation

_cache = {}


# ---------------------------------------------------------------- packing
def _pack(edge_index, batch, n, g):
    """Host-side graph preprocessing: per-core slot streams + schedule."""
    nsh = n // NC
    nsup = -(-nsh // SUP)
    nshp = nsup * SUP
    npad = NC * nshp
    nchunks = -(-npad // CHUNK)

    src = np.asarray(edge_index[0], dtype=np.int64)
    dst = np.asarray(edge_index[1], dtype=np.int64)
    batch = np.asarray(batch, dtype=np.int64)

    deg = (np.bincount(dst, minlength=n) + 1).astype(np.float32)
    dinv = deg ** np.float32(-0.5)

    r_src = nshp * (src // nsh) + src % nsh          # padded table rows
    core = dst // nsh
    dstloc = dst % nsh
    sup_of = dstloc // SUP
    chunk_of = r_src // CHUNK

    w_of = (dstloc % SUP) >= 128
    key = (core * nsup + sup_of) * nchunks + chunk_of
    nk = NC * nsup * nchunks
    cnt = np.bincount(key, minlength=nk).reshape(NC, nsup, nchunks)
    cnt_w1 = np.bincount(key[w_of], minlength=nk).reshape(NC, nsup, nchunks)
    cnt_w0 = cnt - cnt_w1

    b_tiles = [-(-int(cnt[:, :, c].max()) // 128) for c in range(nchunks)]
    lo_t, hi_t = [], []
    for c in range(nchunks):
        if b_tiles[c] == 0:
            lo_t.append(0)
            hi_t.append(0)
        else:
            lo_t.append(int(cnt_w0[:, :, c].min()) // 128)
            hi_t.append(min(-(-int(cnt_w0[:, :, c].max()) // 128), b_tiles[c]))
    et = int(sum(b_tiles))
    tt = et + 2
    offs = np.concatenate([[0], np.cumsum(b_tiles)]).astype(np.int64)

    sched = []                       # (tile, w) — uniform across cores/supers
    for c in range(nchunks):
        for t in range(b_tiles[c]):
            tg = int(offs[c]) + t
            if t < hi_t[c]:
                sched.append((tg, 0))
            if t >= lo_t[c]:
                sched.append((tg, 1))
    sched.append((tt - 2, 0))
    sched.append((tt - 1, 1))

    calls = []                       # (chunk, tile_off, n_tiles)
    for c in range(nchunks):
        t = 0
        while t < b_tiles[c]:
            k = min(CALL_TILES, b_tiles[c] - t)
            calls.append((c, int(offs[c]) + t, k))
            t += k

    es = et * 128
    order = np.lexsort((dstloc, chunk_of, sup_of, core))
    so, do, co, ko, ro = (src[order], dstloc[order] % SUP, chunk_of[order],
                          core[order], r_src[order])
    su = sup_of[order]
    normv = (dinv[so] * dinv[dst[order]]).astype(np.float32)

    gkey = (ko * nsup + su) * nchunks + co
    gstart = np.zeros(nk + 1, np.int64)
    np.add.at(gstart, gkey + 1, 1)
    gstart = np.cumsum(gstart)
    within = np.arange(len(so)) - gstart[gkey]
    slot = offs[co] * 128 + within
    p_ = slot % 128
    t_ = slot // 128

    idx_flat = np.zeros((NC, nsup, max(es, 16)), np.int16)
    idx_flat[ko, su, slot] = (ro % CHUNK).astype(np.int16)
    IDX = np.tile(
        idx_flat[:, :, :es].reshape(NC, nsup, es // 16, 16).transpose(0, 1, 3, 2),
        (1, 1, 8, 1)).copy()
    SN = np.zeros((NC, nsup, 128, tt, 2), np.float32)
    SN[..., 0] = 999.0
    SN[ko, su, p_, t_, 0] = do.astype(np.float32)
    SN[ko, su, p_, t_, 1] = normv
    BL = np.full((NC, nsup, 128, 2), 999.0, np.float32)

    ar = np.arange(128)
    for k in range(NC):
        for s in range(nsup):
            g0 = k * nsh + s * SUP
            for half in range(2):
                rows = g0 + 128 * half + ar
                valid = rows < (k + 1) * nsh
                rc = np.minimum(rows, n - 1)
                dv = np.where(valid, dinv[rc], 0.0)
                SN[k, s, :, tt - 2 + half, 0] = 128 * half + ar
                SN[k, s, :, tt - 2 + half, 1] = (dv * dv).astype(np.float32)
                BL[k, s, :, half] = np.where(valid, batch[rc], 999).astype(np.float32)

    cnts = np.bincount(batch, minlength=g).astype(np.float32)
    rcnt = (1.0 / np.maximum(cnts, 1.0)).astype(np.float32).reshape(g, 1)

    return dict(n=n, g=g, nsh=nsh, nsup=nsup, nshp=nshp, npad=npad,
                nchunks=nchunks, b_tiles=tuple(b_tiles), et=et, tt=tt,
                sched=tuple(sched), calls=tuple(calls), es=es,
                IDX=IDX, SN=SN, BL=BL, rcnt=rcnt)


# ---------------------------------------------------------------- program
def _build(pk):
    import concourse.bacc as bacc
    import concourse.mybir as mybir
    import concourse.tile as tile
    from concourse.library_config import mlp as mlp_lib

    f32 = mybir.dt.float32
    nsup, tt, npad, nshp = pk["nsup"], pk["tt"], pk["npad"], pk["nshp"]
    es, g = pk["es"], pk["g"]
    sched = pk["sched"]

    nc = bacc.Bacc("TRN2", target_bir_lowering=False, debug=False,
                   num_devices=NC, num_swdge_queues=NQ)

    xpad = nc.dram_tensor("xpad", [npad, F], f32, kind="ExternalInput")
    xown = nc.dram_tensor("xown", [nshp, F], f32, kind="ExternalInput")
    IDXd = nc.dram_tensor("IDX", [nsup, 128, es // 16], mybir.dt.int16,
                          kind="ExternalInput")
    SNd = nc.dram_tensor("SN", [nsup, 128, tt * 2], f32, kind="ExternalInput")
    BLd = nc.dram_tensor("BL", [nsup, 128, 2], f32, kind="ExternalInput")
    Wd = {}
    for nm, shp in [("aW1", [F, H]), ("cW1", [F, H]), ("aW2", [H, H]),
                    ("cW2", [H, H]), ("mW", [H, A]), ("f1W", [H, 64]),
                    ("f2W", [64, 1]), ("ab1", [H, 1]), ("cb1", [H, 1]),
                    ("ab2", [H, 1]), ("cb2", [H, 1]), ("mb", [A, 1]),
                    ("f1b", [64, 1]), ("f2b", [1, 1]), ("logstd", [1, A]),
                    ("rcnt", [g, 1]), ("iota_lo", [128, 128]),
                    ("iota_hi", [128, 128]), ("giota", [128, g]),
                    ("ident", [128, 128])]:
        Wd[nm] = nc.dram_tensor(nm, shp, f32, kind="ExternalInput")

    mean_out = nc.dram_tensor("mean_out", [nshp, A], f32, kind="ExternalOutput")
    value_out = nc.dram_tensor("value_out", [1, g], f32, kind="ExternalOutput")
    std_out = nc.dram_tensor("std_out", [1, A], f32, kind="ExternalOutput")

    w_first = {w: min(i for i, (_, ww) in enumerate(sched) if ww == w) for w in (0, 1)}
    w_last = {w: max(i for i, (_, ww) in enumerate(sched) if ww == w) for w in (0, 1)}

    eq, mu = mybir.AluOpType.is_equal, mybir.AluOpType.mult
    RELU = mybir.ActivationFunctionType.Relu
    TANH = mybir.ActivationFunctionType.Tanh
    EXP = mybir.ActivationFunctionType.Exp

    with tile.TileContext(nc) as tc:
        nc.gpsimd.load_library(mlp_lib)
        ctx = ExitStack()
        cpool = ctx.enter_context(tc.tile_pool(name="consts", bufs=1))
        dram = ctx.enter_context(tc.tile_pool(name="dram", bufs=1, space="DRAM"))

        C = {}
        for nm in Wd:
            t = cpool.tile(list(Wd[nm].shape), f32, name=f"c_{nm}", tag=f"c_{nm}")
            nc.sync.dma_start(t[:], Wd[nm][:])
            C[nm] = t
        iw = [C["iota_lo"], C["iota_hi"]]

        T2own = dram.tile([nshp, 2 * H], f32, name="T2own", tag="T2own")
        T2full = dram.tile([npad, 2 * H], f32, name="T2full", tag="T2full",
                           addr_space="Shared")
        ARin = dram.tile([g, H], f32, name="ARin", tag="ARin")
        ARout = dram.tile([g, H], f32, name="ARout", tag="ARout",
                          addr_space="Shared")

        gq = [0]

        def propagate(sup, table, self_table, elem, gbuf_pool, spool, idxp,
                      snp, psum_segs):
            idxt = idxp.tile([128, es // 16], mybir.dt.int16, name="idxt", tag="idxt")
            nc.sync.dma_start(idxt[:], IDXd[sup, :, :])
            snt = snp.tile([128, tt * 2], f32, name="snt", tag="snt")
            nc.sync.dma_start(snt[:], SNd[sup, :, :])
            gb = gbuf_pool.tile([128, tt, elem], f32, name="gb", tag="gb")
            for (c, t0, ntl) in pk["calls"]:
                nidx = ntl * 128
                lo = c * CHUNK
                hi = min(lo + CHUNK, npad)
                nc.gpsimd.dma_gather(
                    gb[:, t0:t0 + ntl, :], table[lo:hi, :],
                    idxt[:, t0 * 8:t0 * 8 + nidx // 16],
                    nidx, nidx, elem, queue_num=gq[0] % NQ)
                gq[0] += 1
            r0 = sup * SUP
            nc.sync.dma_start(
                gb[:, tt - 2:tt, :],
                self_table[r0:r0 + SUP, :].rearrange("(j p) e -> p j e", p=128))
            nmm = elem // 128
            pss = [[psum_segs.tile([128, 128], f32, name=f"ps{m}{w}",
                                   tag=f"ps{m}{w}")
                    for w in (0, 1)] for m in range(nmm)]
            for i, (t, w) in enumerate(sched):
                S = spool.tile([128, 128], f32, name="S", tag="S")
                nc.vector.tensor_scalar(S[:], iw[w][:], snt[:, 2 * t:2 * t + 1],
                                        snt[:, 2 * t + 1:2 * t + 2], eq, mu)
                st, sp = i == w_first[w], i == w_last[w]
                for m in range(nmm):
                    nc.tensor.matmul(pss[m][w][:], lhsT=gb[:, t, 128 * m:128 * (m + 1)],
                                     rhs=S[:], start=st, stop=sp)
            return pss

        # ------- phase 1: propagate x; layer-1 MLPs; build T2own
        with tc.tile_pool(name="g1", bufs=2) as gp1, \
                tc.tile_pool(name="s1", bufs=6) as sp1, \
                tc.tile_pool(name="ix1", bufs=2) as ixp, \
                tc.tile_pool(name="sn1", bufs=2) as snp, \
                tc.tile_pool(name="pseg1", bufs=2, space="PSUM") as psg, \
                tc.tile_pool(name="pwork1", bufs=3, space="PSUM") as pwk, \
                tc.tile_pool(name="e1", bufs=3) as ep:
            for sup in range(nsup):
                pss = propagate(sup, xpad, xown, F, gp1, sp1, ixp, snp, psg)
                for w in (0, 1):
                    P = ep.tile([128, 128], f32, name="P", tag="P")
                    nc.vector.tensor_copy(P[:], pss[0][w][:])
                    rows0 = sup * SUP + w * 128
                    for Wn, bn, col in (("aW1", "ab1", 0), ("cW1", "cb1", H)):
                        z = pwk.tile([128, 128], f32, name="z", tag="wk")
                        nc.tensor.matmul(z[:], lhsT=C[Wn][:], rhs=P[:],
                                         start=True, stop=True)
                        act = ep.tile([128, 128], f32, name="act", tag="act")
                        nc.scalar.activation(act[:], z[:], RELU, bias=C[bn][:])
                        zt = pwk.tile([128, 128], f32, name="zt", tag="wk")
                        nc.tensor.transpose(zt[:], act[:], C["ident"][:])
                        nm_ = ep.tile([128, 128], f32, name="nm", tag="nm")
                        nc.vector.tensor_copy(nm_[:], zt[:])
                        nc.sync.dma_start(T2own[rows0:rows0 + 128, col:col + H],
                                          nm_[:])

        # ------- phase 2: exchange layer-2 table
        nc.gpsimd.collective_compute(
            "AllGather", mybir.AluOpType.bypass,
            replica_groups=[list(range(NC))],
            ins=[T2own.opt()], outs=[T2full.opt()])

        # ------- phase 3: propagate [a1|c1]; heads; pooled partial sums
        with tc.tile_pool(name="g2", bufs=2) as gp2, \
                tc.tile_pool(name="s2", bufs=6) as sp2, \
                tc.tile_pool(name="ix2", bufs=2) as ixp2, \
                tc.tile_pool(name="sn2", bufs=2) as snp2, \
                tc.tile_pool(name="blp", bufs=2) as blp, \
                tc.tile_pool(name="pseg2", bufs=1, space="PSUM") as psg2, \
                tc.tile_pool(name="ppool", bufs=1, space="PSUM") as ppl, \
                tc.tile_pool(name="pwork2", bufs=3, space="PSUM") as pwk2, \
                tc.tile_pool(name="e2", bufs=3) as ep2:
            pool_ps = ppl.tile([g, H], f32, name="poolps", tag="poolps")
            for sup in range(nsup):
                blt = blp.tile([128, 2], f32, name="blt", tag="blt")
                nc.sync.dma_start(blt[:], BLd[sup, :, :])
                pss = propagate(sup, T2full, T2own, 2 * H, gp2, sp2, ixp2,
                                snp2, psg2)
                for w in (0, 1):
                    P2a = ep2.tile([128, 128], f32, name="P2a", tag="P2a")
                    nc.vector.tensor_copy(P2a[:], pss[0][w][:])
                    P2c = ep2.tile([128, 128], f32, name="P2c", tag="P2c")
                    nc.vector.tensor_copy(P2c[:], pss[1][w][:])
                    rows0 = sup * SUP + w * 128
                    z = pwk2.tile([128, 128], f32, name="z2", tag="wk2")
                    nc.tensor.matmul(z[:], lhsT=C["aW2"][:], rhs=P2a[:],
                                     start=True, stop=True)
                    a2 = ep2.tile([128, 128], f32, name="a2", tag="a2")
                    nc.scalar.activation(a2[:], z[:], RELU, bias=C["ab2"][:])
                    zm = pwk2.tile([A, 128], f32, name="zm", tag="wk2")
                    nc.tensor.matmul(zm[:], lhsT=C["mW"][:], rhs=a2[:],
                                     start=True, stop=True)
                    mt = ep2.tile([A, 128], f32, name="mt", tag="mt")
                    nc.scalar.activation(mt[:], zm[:], TANH, bias=C["mb"][:])
                    mtp = pwk2.tile([128, A], f32, name="mtp", tag="wk2")
                    nc.tensor.transpose(mtp[:], mt[:], C["ident"][:A, :A])
                    mrow = ep2.tile([128, A], f32, name="mrow", tag="mrow")
                    nc.vector.tensor_copy(mrow[:], mtp[:])
                    nc.sync.dma_start(mean_out[rows0:rows0 + 128, :], mrow[:])
                    zc = pwk2.tile([128, 128], f32, name="zc2", tag="wk2")
                    nc.tensor.matmul(zc[:], lhsT=C["cW2"][:], rhs=P2c[:],
                                     start=True, stop=True)
                    c2 = ep2.tile([128, 128], f32, name="c2", tag="c2")
                    nc.scalar.activation(c2[:], zc[:], RELU, bias=C["cb2"][:])
                    c2tp = pwk2.tile([128, 128], f32, name="c2tp", tag="wk2")
                    nc.tensor.transpose(c2tp[:], c2[:], C["ident"][:])
                    c2n = ep2.tile([128, 128], f32, name="c2n", tag="c2n")
                    nc.vector.tensor_copy(c2n[:], c2tp[:])
                    Sg = ep2.tile([128, g], f32, name="Sg", tag="Sg")
                    nc.vector.tensor_scalar(Sg[:], C["giota"][:], blt[:, w:w + 1],
                                            None, eq)
                    nc.tensor.matmul(pool_ps[:], lhsT=Sg[:], rhs=c2n[:],
                                     start=(sup == 0 and w == 0),
                                     stop=(sup == nsup - 1 and w == 1))
            pool_sb = ep2.tile([g, H], f32, name="pool_sb", tag="pool_sb")
            nc.vector.tensor_copy(pool_sb[:], pool_ps[:])
            nc.sync.dma_start(ARin[:, :], pool_sb[:])

        # ------- phase 4: AllReduce pooled sums; value head; std
        nc.gpsimd.collective_compute(
            "AllReduce", mybir.AluOpType.add,
            replica_groups=[list(range(NC))],
            ins=[ARin.opt()], outs=[ARout.opt()])
        with tc.tile_pool(name="v", bufs=1) as vp, \
                tc.tile_pool(name="pv", bufs=1, space="PSUM") as pv:
            gx = vp.tile([g, H], f32, name="gx", tag="gx")
            nc.sync.dma_start(gx[:], ARout[:, :])
            nc.vector.tensor_scalar(gx[:], gx[:], C["rcnt"][:], None, mu)
            gxt_p = pv.tile([H, g], f32, name="gxt_p", tag="gxt_p")
            nc.tensor.transpose(gxt_p[:], gx[:], C["ident"][:g, :g])
            gxt = vp.tile([H, g], f32, name="gxt", tag="gxt")
            nc.vector.tensor_copy(gxt[:], gxt_p[:])
            z1 = pv.tile([64, g], f32, name="z1", tag="z1")
            nc.tensor.matmul(z1[:], lhsT=C["f1W"][:], rhs=gxt[:],
                             start=True, stop=True)
            v1 = vp.tile([64, g], f32, name="v1", tag="v1")
            nc.scalar.activation(v1[:], z1[:], RELU, bias=C["f1b"][:])
            zv = pv.tile([1, g], f32, name="zv", tag="zv")
            nc.tensor.matmul(zv[:], lhsT=C["f2W"][:], rhs=v1[:],
                             start=True, stop=True)
            vsb = vp.tile([1, g], f32, name="vsb", tag="vsb")
            nc.vector.tensor_scalar(vsb[:], zv[:], C["f2b"][:], None,
                                    mybir.AluOpType.add)
            nc.sync.dma_start(value_out[:, :], vsb[:])
            es_ = vp.tile([1, A], f32, name="es_", tag="es_")
            nc.scalar.activation(es_[:], C["logstd"][:], EXP)
            nc.sync.dma_start(std_out[:, :], es_[:])
        ctx.close()
    nc.compile()
    return nc


# ---------------------------------------------------------------- runner
def _run(nc, pk, inputs):
    from concourse.bass_utils import run_bass_kernel_spmd

    n, g = pk["n"], pk["g"]
    nsh, nshp, npad, nsup, tt = (pk["nsh"], pk["nshp"], pk["npad"],
                                 pk["nsup"], pk["tt"])

    x = np.ascontiguousarray(np.asarray(inputs["x"], np.float32))
    xpad = np.zeros((npad, F), np.float32)
    for k in range(NC):
        xpad[k * nshp:k * nshp + nsh] = x[k * nsh:(k + 1) * nsh]

    iota = np.tile(np.arange(128, dtype=np.float32), (128, 1))
    common = {
        "xpad": xpad,
        "aW1": np.asarray(inputs["aW1"], np.float32),
        "cW1": np.asarray(inputs["cW1"], np.float32),
        "aW2": np.asarray(inputs["aW2"], np.float32),
        "cW2": np.asarray(inputs["cW2"], np.float32),
        "mW": np.asarray(inputs["mW"], np.float32),
        "f1W": np.asarray(inputs["f1W"], np.float32),
        "f2W": np.asarray(inputs["f2W"], np.float32),
        "ab1": np.asarray(inputs["ab1"], np.float32).reshape(H, 1),
        "cb1": np.asarray(inputs["cb1"], np.float32).reshape(H, 1),
        "ab2": np.asarray(inputs["ab2"], np.float32).reshape(H, 1),
        "cb2": np.asarray(inputs["cb2"], np.float32).reshape(H, 1),
        "mb": np.asarray(inputs["mb"], np.float32).reshape(A, 1),
        "f1b": np.asarray(inputs["f1b"], np.float32).reshape(64, 1),
        "f2b": np.asarray(inputs["f2b"], np.float32).reshape(1, 1),
        "logstd": np.asarray(inputs["log_std"], np.float32).reshape(1, A),
        "rcnt": pk["rcnt"],
        "iota_lo": iota,
        "iota_hi": iota + np.float32(128.0),
        "giota": np.tile(np.arange(g, dtype=np.float32), (128, 1)),
        "ident": np.eye(128, dtype=np.float32),
    }
    in_maps = []
    for k in range(NC):
        m = dict(common)
        m["xown"] = np.ascontiguousarray(xpad[k * nshp:(k + 1) * nshp])
        m["IDX"] = pk["IDX"][k]
        m["SN"] = np.ascontiguousarray(pk["SN"][k].reshape(nsup, 128, tt * 2))
        m["BL"] = pk["BL"][k]
        in_maps.append(m)

    res = run_bass_kernel_spmd(nc, in_maps, core_ids=list(range(NC)))
    _last.update(nc=nc, in_maps=in_maps, pk=pk)
    mean = np.concatenate([res.results[k]["mean_out"][:nsh] for k in range(NC)])
    value = res.results[0]["value_out"].reshape(g, 1)
    std = np.broadcast_to(res.results[0]["std_out"].reshape(1, A), (n, A)).copy()
    return mean, std, value


def kernel(**inputs):
    n = int(np.asarray(inputs["x"]).shape[0])
    g = G
    pk = _pack(np.asarray(inputs["edge_index"]), inputs["batch"], n, g)
    key = (n, g, pk["b_tiles"], pk["sched"])
    if key not in _cache:
        _cache[key] = _build(pk)
    return _run(_cache[key], pk, inputs)


# ------------------------------------------------- timing (test-only helper)
_last = {}


def _make_runner(nc, in_maps):
    """Jitted shard_map runner with device-resident inputs (axon path)."""
    import jax
    from jax.experimental.shard_map import shard_map
    from jax.sharding import Mesh, PartitionSpec

    import concourse.mybir as mybir
    from concourse.bass2jax import (_bass_exec_p, install_neuronx_cc_hook,
                                    partition_id_tensor)

    install_neuronx_cc_hook()
    in_names, out_names, out_avals, zero_outs = [], [], [], []
    pname = nc.partition_id_tensor.name if nc.partition_id_tensor else None
    for alloc in nc.m.functions[0].allocations:
        if not isinstance(alloc, mybir.MemoryLocationSet):
            continue
        name = alloc.memorylocations[0].name
        if alloc.kind == "ExternalInput":
            if name != pname:
                in_names.append(name)
        elif alloc.kind == "ExternalOutput":
            shape = tuple(alloc.tensor_shape)
            dtype = mybir.dt.np(alloc.dtype)
            out_names.append(name)
            out_avals.append(jax.core.ShapedArray(shape, dtype))
            zero_outs.append(np.zeros(shape, dtype))
    all_in = in_names + out_names + ([pname] if pname else [])

    def _body(*args):
        operands = list(args)
        if pname:
            operands.append(partition_id_tensor())
        return tuple(_bass_exec_p.bind(
            *operands, out_avals=tuple(out_avals), in_names=tuple(all_in),
            out_names=tuple(out_names), lowering_input_output_aliases=(),
            sim_require_finite=True, sim_require_nnan=True, nc=nc))

    ncor = len(in_maps)
    mesh = Mesh(np.asarray(jax.devices()[:ncor]), ("core",))
    specs_in = (PartitionSpec("core"),) * (len(in_names) + len(out_names))
    jf = jax.jit(shard_map(_body, mesh=mesh, in_specs=specs_in,
                           out_specs=(PartitionSpec("core"),) * len(out_names),
                           check_rep=False), keep_unused=True)
    dev_in = [jax.device_put(np.concatenate(
        [np.asarray(in_maps[c][nm]) for c in range(ncor)], axis=0))
        for nm in in_names]
    dev_zero = [jax.device_put(np.zeros((ncor * z.shape[0], *z.shape[1:]),
                                        z.dtype)) for z in zero_outs]

    def run():
        outs = jf(*dev_in, *dev_zero)
        jax.block_until_ready(outs)
        return outs

    return run


def _build_null(pk):
    """Same external I/O as the real program, near-empty body."""
    import concourse.bacc as bacc
    import concourse.mybir as mybir
    import concourse.tile as tile

    f32 = mybir.dt.float32
    nsup, tt, npad, nshp, es, g = (pk["nsup"], pk["tt"], pk["npad"],
                                   pk["nshp"], pk["es"], pk["g"])
    nc = bacc.Bacc("TRN2", target_bir_lowering=False, debug=False,
                   num_devices=NC, num_swdge_queues=NQ)
    nc.dram_tensor("xpad", [npad, F], f32, kind="ExternalInput")
    nc.dram_tensor("xown", [nshp, F], f32, kind="ExternalInput")
    nc.dram_tensor("IDX", [nsup, 128, es // 16], mybir.dt.int16, kind="ExternalInput")
    nc.dram_tensor("SN", [nsup, 128, tt * 2], f32, kind="ExternalInput")
    nc.dram_tensor("BL", [nsup, 128, 2], f32, kind="ExternalInput")
    names = [("aW1", [F, H]), ("cW1", [F, H]), ("aW2", [H, H]), ("cW2", [H, H]),
             ("mW", [H, A]), ("f1W", [H, 64]), ("f2W", [64, 1]), ("ab1", [H, 1]),
             ("cb1", [H, 1]), ("ab2", [H, 1]), ("cb2", [H, 1]), ("mb", [A, 1]),
             ("f1b", [64, 1]), ("f2b", [1, 1]), ("logstd", [1, A]),
             ("rcnt", [g, 1]), ("iota_lo", [128, 128]), ("iota_hi", [128, 128]),
             ("giota", [128, g]), ("ident", [128, 128])]
    ten = {nm: nc.dram_tensor(nm, shp, f32, kind="ExternalInput")
           for nm, shp in names}
    mo = nc.dram_tensor("mean_out", [nshp, A], f32, kind="ExternalOutput")
    vo = nc.dram_tensor("value_out", [1, g], f32, kind="ExternalOutput")
    so = nc.dram_tensor("std_out", [1, A], f32, kind="ExternalOutput")
    with tile.TileContext(nc) as tc:
        with tc.tile_pool(name="p", bufs=1) as pool:
            t = pool.tile([1, A], f32, name="t", tag="t")
            nc.sync.dma_start(t[:], ten["logstd"][:])
            nc.sync.dma_start(so[:, :], t[:])
            t2 = pool.tile([1, g], f32, name="t2", tag="t2")
            nc.sync.dma_start(t2[:], ten["rcnt"][:].rearrange("a b -> b a"))
            nc.sync.dma_start(vo[:, :], t2[:])
            t3 = pool.tile([128, A], f32, name="t3", tag="t3")
            nc.sync.dma_start(t3[:], ten["giota"][:, :A])
            nc.sync.dma_start(mo[:128, :], t3[:])
    nc.compile()
    return nc


def measure_exec_ns(iters=8):
    import time
    if "nc" not in _last:
        return None
    run_full = _make_runner(_last["nc"], _last["in_maps"])
    run_null = _make_runner(_build_null(_last["pk"]), _last["in_maps"])

    def tmin(run):
        for _ in range(2):
            run()
        ts = []
        for _ in range(iters):
            t0 = time.perf_counter()
            run()
            ts.append(time.perf_counter() - t0)
        return min(ts), ts

    tf, raw_f = tmin(run_full)
    tn, raw_n = tmin(run_null)
    print(f"  full: {[f'{x * 1e3:.0f}' for x in raw_f]}  "
          f"null: {[f'{x * 1e3:.0f}' for x in raw_n]}")
    return int((tf - tn) * 1e9)


# revision 5
# speedup vs baseline: 1.1442x; 1.1442x over previous
"""ActorCriticGNN MAPPO forward on 8 Trainium2 NeuronCores (Bass/Tile).

Strategy
--------
GCNConv(x, W, b) = A_hat @ (x W) + b with A_hat = D^-1/2 (A+I) D^-1/2, and
A_hat @ (x W) = (A_hat @ x) W, so each conv is: sparse propagation, then a
dense 128x128 matmul. Actor and critic layer-1 share the propagation of x
(one pass), and layer-2 actor/critic propagations fuse into one 256-wide
pass over the concatenated table [a1|c1]. Only 2 sparse passes total.

Sharding: nodes (and their in-edges) are range-partitioned across the 8
cores. The layer-1 table is the replicated input x; the layer-2 table is
exchanged with one AllGather. Per-graph pooled sums use an AllReduce.

Per core, each propagation processes its ~200K in-edges in "supers" of 256
destination nodes. Edge source rows are fetched with dma_gather (int16
indices -> 4 chunk sub-tables of <=32768 rows). The segment sum runs on the
tensor engine: for each 128-slot K-tile, a one-hot matrix
S[slot, dst] = (iota == dstlocal[slot]) * norm[slot] is built in a single
DVE tensor_scalar op, and psum[feat, dst] += gathered_tile.T @ S
accumulates the normalized sums. norm = dinv[src]*dinv[dst] (dinv[dst]^2 for
the self-loop slots) carries the full GCN normalization, so tables are
gathered raw. All per-edge index math (sorting, padding, norm values) is
host-side preprocessing; all FLOPs on features run on device.
"""
from contextlib import ExitStack

import numpy as np

# ---------------------------------------------------------------- config
F, H, A, G, NC = 128, 128, 8, 64, 8
SUP = 256            # destination nodes per super-group
CHUNK = 32768        # gather sub-table rows (int16 index reach)
CALL_TILES = 6       # <=768 indices per dma_gather call (HW limit ~1K)
NQ = 4               # SWDGE queues for gather descriptor generation
BF16_T2 = True       # layer-2 table (a1|c1) + its S matrices in bf16

_cache = {}


# ---------------------------------------------------------------- packing
def _pack(edge_index, batch, n, g):
    """Host-side graph preprocessing: per-core slot streams + schedule."""
    nsh = n // NC
    nsup = -(-nsh // SUP)
    nshp = nsup * SUP
    npad = NC * nshp
    nchunks = -(-npad // CHUNK)

    src = np.asarray(edge_index[0], dtype=np.int64)
    dst = np.asarray(edge_index[1], dtype=np.int64)
    batch = np.asarray(batch, dtype=np.int64)

    deg = (np.bincount(dst, minlength=n) + 1).astype(np.float32)
    dinv = deg ** np.float32(-0.5)

    r_src = nshp * (src // nsh) + src % nsh          # padded table rows
    core = dst // nsh
    dstloc = dst % nsh
    sup_of = dstloc // SUP
    chunk_of = r_src // CHUNK

    w_of = (dstloc % SUP) >= 128
    key = (core * nsup + sup_of) * nchunks + chunk_of
    nk = NC * nsup * nchunks
    cnt = np.bincount(key, minlength=nk).reshape(NC, nsup, nchunks)
    cnt_w1 = np.bincount(key[w_of], minlength=nk).reshape(NC, nsup, nchunks)
    cnt_w0 = cnt - cnt_w1

    b_tiles = [-(-int(cnt[:, :, c].max()) // 128) for c in range(nchunks)]
    lo_t, hi_t = [], []
    for c in range(nchunks):
        if b_tiles[c] == 0:
            lo_t.append(0)
            hi_t.append(0)
        else:
            lo_t.append(int(cnt_w0[:, :, c].min()) // 128)
            hi_t.append(min(-(-int(cnt_w0[:, :, c].max()) // 128), b_tiles[c]))
    et = int(sum(b_tiles))
    tt = et + 2
    offs = np.concatenate([[0], np.cumsum(b_tiles)]).astype(np.int64)

    sched = []                       # (tile, w) — uniform across cores/supers
    for c in range(nchunks):
        for t in range(b_tiles[c]):
            tg = int(offs[c]) + t
            if t < hi_t[c]:
                sched.append((tg, 0))
            if t >= lo_t[c]:
                sched.append((tg, 1))
    sched.append((tt - 2, 0))
    sched.append((tt - 1, 1))

    calls = []                       # (chunk, tile_off, n_tiles)
    for c in range(nchunks):
        t = 0
        while t < b_tiles[c]:
            k = min(CALL_TILES, b_tiles[c] - t)
            calls.append((c, int(offs[c]) + t, k))
            t += k

    es = et * 128
    order = np.lexsort((dstloc, chunk_of, sup_of, core))
    so, do, co, ko, ro = (src[order], dstloc[order] % SUP, chunk_of[order],
                          core[order], r_src[order])
    su = sup_of[order]
    normv = (dinv[so] * dinv[dst[order]]).astype(np.float32)

    gkey = (ko * nsup + su) * nchunks + co
    gstart = np.zeros(nk + 1, np.int64)
    np.add.at(gstart, gkey + 1, 1)
    gstart = np.cumsum(gstart)
    within = np.arange(len(so)) - gstart[gkey]
    slot = offs[co] * 128 + within
    p_ = slot % 128
    t_ = slot // 128

    idx_flat = np.zeros((NC, nsup, max(es, 16)), np.int16)
    idx_flat[ko, su, slot] = (ro % CHUNK).astype(np.int16)
    IDX = np.tile(
        idx_flat[:, :, :es].reshape(NC, nsup, es // 16, 16).transpose(0, 1, 3, 2),
        (1, 1, 8, 1)).copy()
    SN = np.zeros((NC, nsup, 128, tt, 2), np.float32)
    SN[..., 0] = 999.0
    SN[ko, su, p_, t_, 0] = do.astype(np.float32)
    SN[ko, su, p_, t_, 1] = normv
    BL = np.full((NC, nsup, 128, 2), 999.0, np.float32)

    ar = np.arange(128)
    for k in range(NC):
        for s in range(nsup):
            g0 = k * nsh + s * SUP
            for half in range(2):
                rows = g0 + 128 * half + ar
                valid = rows < (k + 1) * nsh
                rc = np.minimum(rows, n - 1)
                dv = np.where(valid, dinv[rc], 0.0)
                SN[k, s, :, tt - 2 + half, 0] = 128 * half + ar
                SN[k, s, :, tt - 2 + half, 1] = (dv * dv).astype(np.float32)
                BL[k, s, :, half] = np.where(valid, batch[rc], 999).astype(np.float32)

    cnts = np.bincount(batch, minlength=g).astype(np.float32)
    rcnt = (1.0 / np.maximum(cnts, 1.0)).astype(np.float32).reshape(g, 1)

    return dict(n=n, g=g, nsh=nsh, nsup=nsup, nshp=nshp, npad=npad,
                nchunks=nchunks, b_tiles=tuple(b_tiles), et=et, tt=tt,
                sched=tuple(sched), calls=tuple(calls), es=es,
                IDX=IDX, SN=SN, BL=BL, rcnt=rcnt)


# ---------------------------------------------------------------- program
def _build(pk):
    import concourse.bacc as bacc
    import concourse.mybir as mybir
    import concourse.tile as tile
    from concourse.library_config import mlp as mlp_lib

    f32 = mybir.dt.float32
    dt2 = mybir.dt.bfloat16 if BF16_T2 else f32
    nsup, tt, npad, nshp = pk["nsup"], pk["tt"], pk["npad"], pk["nshp"]
    es, g = pk["es"], pk["g"]
    sched = pk["sched"]

    nc = bacc.Bacc("TRN2", target_bir_lowering=False, debug=False,
                   num_devices=NC, num_swdge_queues=NQ)

    xpad = nc.dram_tensor("xpad", [npad, F], f32, kind="ExternalInput")
    xown = nc.dram_tensor("xown", [nshp, F], f32, kind="ExternalInput")
    IDXd = nc.dram_tensor("IDX", [nsup, 128, es // 16], mybir.dt.int16,
                          kind="ExternalInput")
    SNd = nc.dram_tensor("SN", [nsup, 128, tt * 2], f32, kind="ExternalInput")
    BLd = nc.dram_tensor("BL", [nsup, 128, 2], f32, kind="ExternalInput")
    Wd = {}
    for nm, shp in [("aW1", [F, H]), ("cW1", [F, H]), ("aW2", [H, H]),
                    ("cW2", [H, H]), ("mW", [H, A]), ("f1W", [H, 64]),
                    ("f2W", [64, 1]), ("ab1", [H, 1]), ("cb1", [H, 1]),
                    ("ab2", [H, 1]), ("cb2", [H, 1]), ("mb", [A, 1]),
                    ("f1b", [64, 1]), ("f2b", [1, 1]), ("logstd", [1, A]),
                    ("rcnt", [g, 1]), ("iota_lo", [128, 128]),
                    ("iota_hi", [128, 128]), ("giota", [128, g]),
                    ("ident", [128, 128])]:
        Wd[nm] = nc.dram_tensor(nm, shp, f32, kind="ExternalInput")

    mean_out = nc.dram_tensor("mean_out", [nshp, A], f32, kind="ExternalOutput")
    value_out = nc.dram_tensor("value_out", [1, g], f32, kind="ExternalOutput")
    std_out = nc.dram_tensor("std_out", [1, A], f32, kind="ExternalOutput")

    w_first = {w: min(i for i, (_, ww) in enumerate(sched) if ww == w) for w in (0, 1)}
    w_last = {w: max(i for i, (_, ww) in enumerate(sched) if ww == w) for w in (0, 1)}

    eq, mu = mybir.AluOpType.is_equal, mybir.AluOpType.mult
    RELU = mybir.ActivationFunctionType.Relu
    TANH = mybir.ActivationFunctionType.Tanh
    EXP = mybir.ActivationFunctionType.Exp

    with tile.TileContext(nc) as tc:
        nc.gpsimd.load_library(mlp_lib)
        ctx = ExitStack()
        cpool = ctx.enter_context(tc.tile_pool(name="consts", bufs=1))
        dram = ctx.enter_context(tc.tile_pool(name="dram", bufs=1, space="DRAM"))

        C = {}
        for nm in Wd:
            t = cpool.tile(list(Wd[nm].shape), f32, name=f"c_{nm}", tag=f"c_{nm}")
            nc.sync.dma_start(t[:], Wd[nm][:])
            C[nm] = t
        iw = [C["iota_lo"], C["iota_hi"]]

        T2own = dram.tile([nshp, 2 * H], dt2, name="T2own", tag="T2own")
        T2full = dram.tile([npad, 2 * H], dt2, name="T2full", tag="T2full",
                           addr_space="Shared")
        ARin = dram.tile([g, H], f32, name="ARin", tag="ARin")
        ARout = dram.tile([g, H], f32, name="ARout", tag="ARout",
                          addr_space="Shared")

        gq = [0]

        def propagate(sup, table, self_table, elem, gbuf_pool, spool, idxp,
                      snp, psum_segs, dt):
            idxt = idxp.tile([128, es // 16], mybir.dt.int16, name="idxt", tag="idxt")
            nc.sync.dma_start(idxt[:], IDXd[sup, :, :])
            snt = snp.tile([128, tt * 2], f32, name="snt", tag="snt")
            nc.sync.dma_start(snt[:], SNd[sup, :, :])
            gb = gbuf_pool.tile([128, tt, elem], dt, name="gb", tag="gb")
            for (c, t0, ntl) in pk["calls"]:
                nidx = ntl * 128
                lo = c * CHUNK
                hi = min(lo + CHUNK, npad)
                nc.gpsimd.dma_gather(
                    gb[:, t0:t0 + ntl, :], table[lo:hi, :],
                    idxt[:, t0 * 8:t0 * 8 + nidx // 16],
                    nidx, nidx, elem, queue_num=gq[0] % NQ)
                gq[0] += 1
            r0 = sup * SUP
            nc.sync.dma_start(
                gb[:, tt - 2:tt, :],
                self_table[r0:r0 + SUP, :].rearrange("(j p) e -> p j e", p=128))
            nmm = elem // 128
            pss = [[psum_segs.tile([128, 128], f32, name=f"ps{m}{w}",
                                   tag=f"ps{m}{w}")
                    for w in (0, 1)] for m in range(nmm)]
            for i, (t, w) in enumerate(sched):
                S = spool.tile([128, 128], dt, name="S", tag="S")
                nc.vector.tensor_scalar(S[:], iw[w][:], snt[:, 2 * t:2 * t + 1],
                                        snt[:, 2 * t + 1:2 * t + 2], eq, mu)
                st, sp = i == w_first[w], i == w_last[w]
                for m in range(nmm):
                    nc.tensor.matmul(pss[m][w][:], lhsT=gb[:, t, 128 * m:128 * (m + 1)],
                                     rhs=S[:], start=st, stop=sp)
            return pss

        # ------- phase 1: propagate x; layer-1 MLPs; build T2own
        with tc.tile_pool(name="g1", bufs=2) as gp1, \
                tc.tile_pool(name="s1", bufs=6) as sp1, \
                tc.tile_pool(name="ix1", bufs=2) as ixp, \
                tc.tile_pool(name="sn1", bufs=2) as snp, \
                tc.tile_pool(name="pseg1", bufs=2, space="PSUM") as psg, \
                tc.tile_pool(name="pwork1", bufs=3, space="PSUM") as pwk, \
                tc.tile_pool(name="e1", bufs=3) as ep:
            for sup in range(nsup):
                pss = propagate(sup, xpad, xown, F, gp1, sp1, ixp, snp, psg, f32)
                for w in (0, 1):
                    P = ep.tile([128, 128], f32, name="P", tag="P")
                    nc.vector.tensor_copy(P[:], pss[0][w][:])
                    rows0 = sup * SUP + w * 128
                    for Wn, bn, col in (("aW1", "ab1", 0), ("cW1", "cb1", H)):
                        z = pwk.tile([128, 128], f32, name="z", tag="wk")
                        nc.tensor.matmul(z[:], lhsT=C[Wn][:], rhs=P[:],
                                         start=True, stop=True)
                        act = ep.tile([128, 128], f32, name="act", tag="act")
                        nc.scalar.activation(act[:], z[:], RELU, bias=C[bn][:])
                        zt = pwk.tile([128, 128], f32, name="zt", tag="wk")
                        nc.tensor.transpose(zt[:], act[:], C["ident"][:])
                        nm_ = ep.tile([128, 128], dt2, name="nm", tag="nm")
                        nc.vector.tensor_copy(nm_[:], zt[:])
                        nc.sync.dma_start(T2own[rows0:rows0 + 128, col:col + H],
                                          nm_[:])

        # ------- phase 2: exchange layer-2 table
        nc.gpsimd.collective_compute(
            "AllGather", mybir.AluOpType.bypass,
            replica_groups=[list(range(NC))],
            ins=[T2own.opt()], outs=[T2full.opt()])

        # ------- phase 3: propagate [a1|c1]; heads; pooled partial sums
        with tc.tile_pool(name="g2", bufs=2) as gp2, \
                tc.tile_pool(name="s2", bufs=6) as sp2, \
                tc.tile_pool(name="ix2", bufs=2) as ixp2, \
                tc.tile_pool(name="sn2", bufs=2) as snp2, \
                tc.tile_pool(name="blp", bufs=2) as blp, \
                tc.tile_pool(name="pseg2", bufs=1, space="PSUM") as psg2, \
                tc.tile_pool(name="ppool", bufs=1, space="PSUM") as ppl, \
                tc.tile_pool(name="pwork2", bufs=3, space="PSUM") as pwk2, \
                tc.tile_pool(name="e2", bufs=3) as ep2:
            pool_ps = ppl.tile([g, H], f32, name="poolps", tag="poolps")
            for sup in range(nsup):
                blt = blp.tile([128, 2], f32, name="blt", tag="blt")
                nc.sync.dma_start(blt[:], BLd[sup, :, :])
                pss = propagate(sup, T2full, T2own, 2 * H, gp2, sp2, ixp2,
                                snp2, psg2, dt2)
                for w in (0, 1):
                    P2a = ep2.tile([128, 128], f32, name="P2a", tag="P2a")
                    nc.vector.tensor_copy(P2a[:], pss[0][w][:])
                    P2c = ep2.tile([128, 128], f32, name="P2c", tag="P2c")
                    nc.vector.tensor_copy(P2c[:], pss[1][w][:])
                    rows0 = sup * SUP + w * 128
                    z = pwk2.tile([128, 128], f32, name="z2", tag="wk2")
                    nc.tensor.matmul(z[:], lhsT=C["aW2"][:], rhs=P2a[:],
                                     start=True, stop=True)
                    a2 = ep2.tile([128, 128], f32, name="a2", tag="a2")
                    nc.scalar.activation(a2[:], z[:], RELU, bias=C["ab2"][:])
                    zm = pwk2.tile([A, 128], f32, name="zm", tag="wk2")
                    nc.tensor.matmul(zm[:], lhsT=C["mW"][:], rhs=a2[:],
                                     start=True, stop=True)
                    mt = ep2.tile([A, 128], f32, name="mt", tag="mt")
                    nc.scalar.activation(mt[:], zm[:], TANH, bias=C["mb"][:])
                    mtp = pwk2.tile([128, A], f32, name="mtp", tag="wk2")
                    nc.tensor.transpose(mtp[:], mt[:], C["ident"][:A, :A])
                    mrow = ep2.tile([128, A], f32, name="mrow", tag="mrow")
                    nc.vector.tensor_copy(mrow[:], mtp[:])
                    nc.sync.dma_start(mean_out[rows0:rows0 + 128, :], mrow[:])
                    zc = pwk2.tile([128, 128], f32, name="zc2", tag="wk2")
                    nc.tensor.matmul(zc[:], lhsT=C["cW2"][:], rhs=P2c[:],
                                     start=True, stop=True)
                    c2 = ep2.tile([128, 128], f32, name="c2", tag="c2")
                    nc.scalar.activation(c2[:], zc[:], RELU, bias=C["cb2"][:])
                    c2tp = pwk2.tile([128, 128], f32, name="c2tp", tag="wk2")
                    nc.tensor.transpose(c2tp[:], c2[:], C["ident"][:])
                    c2n = ep2.tile([128, 128], f32, name="c2n", tag="c2n")
                    nc.vector.tensor_copy(c2n[:], c2tp[:])
                    Sg = ep2.tile([128, g], f32, name="Sg", tag="Sg")
                    nc.vector.tensor_scalar(Sg[:], C["giota"][:], blt[:, w:w + 1],
                                            None, eq)
                    nc.tensor.matmul(pool_ps[:], lhsT=Sg[:], rhs=c2n[:],
                                     start=(sup == 0 and w == 0),
                                     stop=(sup == nsup - 1 and w == 1))
            pool_sb = ep2.tile([g, H], f32, name="pool_sb", tag="pool_sb")
            nc.vector.tensor_copy(pool_sb[:], pool_ps[:])
            nc.sync.dma_start(ARin[:, :], pool_sb[:])

        # ------- phase 4: AllReduce pooled sums; value head; std
        nc.gpsimd.collective_compute(
            "AllReduce", mybir.AluOpType.add,
            replica_groups=[list(range(NC))],
            ins=[ARin.opt()], outs=[ARout.opt()])
        with tc.tile_pool(name="v", bufs=1) as vp, \
                tc.tile_pool(name="pv", bufs=1, space="PSUM") as pv:
            gx = vp.tile([g, H], f32, name="gx", tag="gx")
            nc.sync.dma_start(gx[:], ARout[:, :])
            nc.vector.tensor_scalar(gx[:], gx[:], C["rcnt"][:], None, mu)
            gxt_p = pv.tile([H, g], f32, name="gxt_p", tag="gxt_p")
            nc.tensor.transpose(gxt_p[:], gx[:], C["ident"][:g, :g])
            gxt = vp.tile([H, g], f32, name="gxt", tag="gxt")
            nc.vector.tensor_copy(gxt[:], gxt_p[:])
            z1 = pv.tile([64, g], f32, name="z1", tag="z1")
            nc.tensor.matmul(z1[:], lhsT=C["f1W"][:], rhs=gxt[:],
                             start=True, stop=True)
            v1 = vp.tile([64, g], f32, name="v1", tag="v1")
            nc.scalar.activation(v1[:], z1[:], RELU, bias=C["f1b"][:])
            zv = pv.tile([1, g], f32, name="zv", tag="zv")
            nc.tensor.matmul(zv[:], lhsT=C["f2W"][:], rhs=v1[:],
                             start=True, stop=True)
            vsb = vp.tile([1, g], f32, name="vsb", tag="vsb")
            nc.vector.tensor_scalar(vsb[:], zv[:], C["f2b"][:], None,
                                    mybir.AluOpType.add)
            nc.sync.dma_start(value_out[:, :], vsb[:])
            es_ = vp.tile([1, A], f32, name="es_", tag="es_")
            nc.scalar.activation(es_[:], C["logstd"][:], EXP)
            nc.sync.dma_start(std_out[:, :], es_[:])
        ctx.close()
    nc.compile()
    return nc


# ---------------------------------------------------------------- runner
def _run(nc, pk, inputs):
    from concourse.bass_utils import run_bass_kernel_spmd

    n, g = pk["n"], pk["g"]
    nsh, nshp, npad, nsup, tt = (pk["nsh"], pk["nshp"], pk["npad"],
                                 pk["nsup"], pk["tt"])

    x = np.ascontiguousarray(np.asarray(inputs["x"], np.float32))
    xpad = np.zeros((npad, F), np.float32)
    for k in range(NC):
        xpad[k * nshp:k * nshp + nsh] = x[k * nsh:(k + 1) * nsh]

    iota = np.tile(np.arange(128, dtype=np.float32), (128, 1))
    common = {
        "xpad": xpad,
        "aW1": np.asarray(inputs["aW1"], np.float32),
        "cW1": np.asarray(inputs["cW1"], np.float32),
        "aW2": np.asarray(inputs["aW2"], np.float32),
        "cW2": np.asarray(inputs["cW2"], np.float32),
        "mW": np.asarray(inputs["mW"], np.float32),
        "f1W": np.asarray(inputs["f1W"], np.float32),
        "f2W": np.asarray(inputs["f2W"], np.float32),
        "ab1": np.asarray(inputs["ab1"], np.float32).reshape(H, 1),
        "cb1": np.asarray(inputs["cb1"], np.float32).reshape(H, 1),
        "ab2": np.asarray(inputs["ab2"], np.float32).reshape(H, 1),
        "cb2": np.asarray(inputs["cb2"], np.float32).reshape(H, 1),
        "mb": np.asarray(inputs["mb"], np.float32).reshape(A, 1),
        "f1b": np.asarray(inputs["f1b"], np.float32).reshape(64, 1),
        "f2b": np.asarray(inputs["f2b"], np.float32).reshape(1, 1),
        "logstd": np.asarray(inputs["log_std"], np.float32).reshape(1, A),
        "rcnt": pk["rcnt"],
        "iota_lo": iota,
        "iota_hi": iota + np.float32(128.0),
        "giota": np.tile(np.arange(g, dtype=np.float32), (128, 1)),
        "ident": np.eye(128, dtype=np.float32),
    }
    in_maps = []
    for k in range(NC):
        m = dict(common)
        m["xown"] = np.ascontiguousarray(xpad[k * nshp:(k + 1) * nshp])
        m["IDX"] = pk["IDX"][k]
        m["SN"] = np.ascontiguousarray(pk["SN"][k].reshape(nsup, 128, tt * 2))
        m["BL"] = pk["BL"][k]
        in_maps.append(m)

    res = run_bass_kernel_spmd(nc, in_maps, core_ids=list(range(NC)))
    _last.update(nc=nc, in_maps=in_maps, pk=pk)
    mean = np.concatenate([res.results[k]["mean_out"][:nsh] for k in range(NC)])
    value = res.results[0]["value_out"].reshape(g, 1)
    std = np.broadcast_to(res.results[0]["std_out"].reshape(1, A), (n, A)).copy()
    return mean, std, value


def kernel(**inputs):
    n = int(np.asarray(inputs["x"]).shape[0])
    g = G
    pk = _pack(np.asarray(inputs["edge_index"]), inputs["batch"], n, g)
    key = (n, g, pk["b_tiles"], pk["sched"])
    if key not in _cache:
        _cache[key] = _build(pk)
    return _run(_cache[key], pk, inputs)


# ------------------------------------------------- timing (test-only helper)
_last = {}


def _make_runner(nc, in_maps):
    """Jitted shard_map runner with device-resident inputs (axon path)."""
    import jax
    from jax.experimental.shard_map import shard_map
    from jax.sharding import Mesh, PartitionSpec

    import concourse.mybir as mybir
    from concourse.bass2jax import (_bass_exec_p, install_neuronx_cc_hook,
                                    partition_id_tensor)

    install_neuronx_cc_hook()
    in_names, out_names, out_avals, zero_outs = [], [], [], []
    pname = nc.partition_id_tensor.name if nc.partition_id_tensor else None
    for alloc in nc.m.functions[0].allocations:
        if not isinstance(alloc, mybir.MemoryLocationSet):
            continue
        name = alloc.memorylocations[0].name
        if alloc.kind == "ExternalInput":
            if name != pname:
                in_names.append(name)
        elif alloc.kind == "ExternalOutput":
            shape = tuple(alloc.tensor_shape)
            dtype = mybir.dt.np(alloc.dtype)
            out_names.append(name)
            out_avals.append(jax.core.ShapedArray(shape, dtype))
            zero_outs.append(np.zeros(shape, dtype))
    all_in = in_names + out_names + ([pname] if pname else [])

    def _body(*args):
        operands = list(args)
        if pname:
            operands.append(partition_id_tensor())
        return tuple(_bass_exec_p.bind(
            *operands, out_avals=tuple(out_avals), in_names=tuple(all_in),
            out_names=tuple(out_names), lowering_input_output_aliases=(),
            sim_require_finite=True, sim_require_nnan=True, nc=nc))

    ncor = len(in_maps)
    mesh = Mesh(np.asarray(jax.devices()[:ncor]), ("core",))
    specs_in = (PartitionSpec("core"),) * (len(in_names) + len(out_names))
    jf = jax.jit(shard_map(_body, mesh=mesh, in_specs=specs_in,
                           out_specs=(PartitionSpec("core"),) * len(out_names),
                           check_rep=False), keep_unused=True)
    dev_in = [jax.device_put(np.concatenate(
        [np.asarray(in_maps[c][nm]) for c in range(ncor)], axis=0))
        for nm in in_names]
    dev_zero = [jax.device_put(np.zeros((ncor * z.shape[0], *z.shape[1:]),
                                        z.dtype)) for z in zero_outs]

    def run():
        outs = jf(*dev_in, *dev_zero)
        jax.block_until_ready(outs)
        return outs

    return run


def _build_null(pk):
    """Same external I/O as the real program, near-empty body."""
    import concourse.bacc as bacc
    import concourse.mybir as mybir
    import concourse.tile as tile

    f32 = mybir.dt.float32
    nsup, tt, npad, nshp, es, g = (pk["nsup"], pk["tt"], pk["npad"],
                                   pk["nshp"], pk["es"], pk["g"])
    nc = bacc.Bacc("TRN2", target_bir_lowering=False, debug=False,
                   num_devices=NC, num_swdge_queues=NQ)
    nc.dram_tensor("xpad", [npad, F], f32, kind="ExternalInput")
    nc.dram_tensor("xown", [nshp, F], f32, kind="ExternalInput")
    nc.dram_tensor("IDX", [nsup, 128, es // 16], mybir.dt.int16, kind="ExternalInput")
    nc.dram_tensor("SN", [nsup, 128, tt * 2], f32, kind="ExternalInput")
    nc.dram_tensor("BL", [nsup, 128, 2], f32, kind="ExternalInput")
    names = [("aW1", [F, H]), ("cW1", [F, H]), ("aW2", [H, H]), ("cW2", [H, H]),
             ("mW", [H, A]), ("f1W", [H, 64]), ("f2W", [64, 1]), ("ab1", [H, 1]),
             ("cb1", [H, 1]), ("ab2", [H, 1]), ("cb2", [H, 1]), ("mb", [A, 1]),
             ("f1b", [64, 1]), ("f2b", [1, 1]), ("logstd", [1, A]),
             ("rcnt", [g, 1]), ("iota_lo", [128, 128]), ("iota_hi", [128, 128]),
             ("giota", [128, g]), ("ident", [128, 128])]
    ten = {nm: nc.dram_tensor(nm, shp, f32, kind="ExternalInput")
           for nm, shp in names}
    mo = nc.dram_tensor("mean_out", [nshp, A], f32, kind="ExternalOutput")
    vo = nc.dram_tensor("value_out", [1, g], f32, kind="ExternalOutput")
    so = nc.dram_tensor("std_out", [1, A], f32, kind="ExternalOutput")
    with tile.TileContext(nc) as tc:
        with tc.tile_pool(name="p", bufs=1) as pool:
            t = pool.tile([1, A], f32, name="t", tag="t")
            nc.sync.dma_start(t[:], ten["logstd"][:])
            nc.sync.dma_start(so[:, :], t[:])
            t2 = pool.tile([1, g], f32, name="t2", tag="t2")
            nc.sync.dma_start(t2[:], ten["rcnt"][:].rearrange("a b -> b a"))
            nc.sync.dma_start(vo[:, :], t2[:])
            t3 = pool.tile([128, A], f32, name="t3", tag="t3")
            nc.sync.dma_start(t3[:], ten["giota"][:, :A])
            nc.sync.dma_start(mo[:128, :], t3[:])
    nc.compile()
    return nc


def measure_exec_ns(iters=8):
    import time
    if "nc" not in _last:
        return None
    run_full = _make_runner(_last["nc"], _last["in_maps"])
    run_null = _make_runner(_build_null(_last["pk"]), _last["in_maps"])

    def tmin(run):
        for _ in range(2):
            run()
        ts = []
        for _ in range(iters):
            t0 = time.perf_counter()
            run()
            ts.append(time.perf_counter() - t0)
        return min(ts), ts

    tf, raw_f = tmin(run_full)
    tn, raw_n = tmin(run_null)
    print(f"  full: {[f'{x * 1e3:.0f}' for x in raw_f]}  "
          f"null: {[f'{x * 1e3:.0f}' for x in raw_n]}")
    return int((tf - tn) * 1e9)


# revision 7
# speedup vs baseline: 1.5421x; 1.3477x over previous
"""ActorCriticGNN MAPPO forward on 8 Trainium2 NeuronCores (Bass/Tile).

Strategy
--------
GCNConv(x, W, b) = A_hat @ (x W) + b with A_hat = D^-1/2 (A+I) D^-1/2, and
A_hat @ (x W) = (A_hat @ x) W, so each conv is: sparse propagation, then a
dense 128x128 matmul. Actor and critic layer-1 share the propagation of x
(one pass), and layer-2 actor/critic propagations fuse into one 256-wide
pass over the concatenated table [a1|c1]. Only 2 sparse passes total.

Sharding: nodes (and their in-edges) are range-partitioned across the 8
cores. The layer-1 table is the replicated input x; the layer-2 table is
exchanged with one AllGather. Per-graph pooled sums use an AllReduce.

Per core, each propagation processes its ~200K in-edges in "supers" of 256
destination nodes. Edge source rows are fetched with dma_gather (int16
indices -> 4 chunk sub-tables of <=32768 rows). The segment sum runs on the
tensor engine: for each 128-slot K-tile, a one-hot matrix
S[slot, dst] = (iota == dstlocal[slot]) * norm[slot] is built in a single
DVE tensor_scalar op, and psum[feat, dst] += gathered_tile.T @ S
accumulates the normalized sums. norm = dinv[src]*dinv[dst] (dinv[dst]^2 for
the self-loop slots) carries the full GCN normalization, so tables are
gathered raw. All per-edge index math (sorting, padding, norm values) is
host-side preprocessing; all FLOPs on features run on device.
"""
from contextlib import ExitStack

import numpy as np

# ---------------------------------------------------------------- config
F, H, A, G, NC = 128, 128, 8, 64, 8
SUP = 256            # destination nodes per super-group
CHUNK = 32768        # gather sub-table rows (int16 index reach)
CALL_TILES = 8       # <=1024 indices per dma_gather call (HW limit ~1K)
NQ = 4               # SWDGE queues for gather descriptor generation
BF16_T2 = True       # layer-2 table (a1|c1) + its S matrices in bf16

_cache = {}


# ---------------------------------------------------------------- packing
def _pack(edge_index, batch, n, g):
    """Host-side graph preprocessing: per-core slot streams + schedule."""
    nsh = n // NC
    nsup = -(-nsh // SUP)
    nshp = nsup * SUP
    npad = NC * nshp
    nchunks = -(-npad // CHUNK)

    src = np.asarray(edge_index[0], dtype=np.int64)
    dst = np.asarray(edge_index[1], dtype=np.int64)
    batch = np.asarray(batch, dtype=np.int64)

    deg = (np.bincount(dst, minlength=n) + 1).astype(np.float32)
    dinv = deg ** np.float32(-0.5)

    r_src = nshp * (src // nsh) + src % nsh          # padded table rows
    core = dst // nsh
    dstloc = dst % nsh
    sup_of = dstloc // SUP
    chunk_of = r_src // CHUNK

    w_of = (dstloc % SUP) >= 128
    key = (core * nsup + sup_of) * nchunks + chunk_of
    nk = NC * nsup * nchunks
    cnt = np.bincount(key, minlength=nk).reshape(NC, nsup, nchunks)
    cnt_w1 = np.bincount(key[w_of], minlength=nk).reshape(NC, nsup, nchunks)
    cnt_w0 = cnt - cnt_w1

    b_tiles = [-(-int(cnt[:, :, c].max()) // 128) for c in range(nchunks)]
    lo_t, hi_t = [], []
    for c in range(nchunks):
        if b_tiles[c] == 0:
            lo_t.append(0)
            hi_t.append(0)
        else:
            lo_t.append(int(cnt_w0[:, :, c].min()) // 128)
            hi_t.append(min(-(-int(cnt_w0[:, :, c].max()) // 128), b_tiles[c]))
    et = int(sum(b_tiles))
    tt = et + 2
    offs = np.concatenate([[0], np.cumsum(b_tiles)]).astype(np.int64)

    sched = []                       # (tile, w) — uniform across cores/supers
    for c in range(nchunks):
        for t in range(b_tiles[c]):
            tg = int(offs[c]) + t
            if t < hi_t[c]:
                sched.append((tg, 0))
            if t >= lo_t[c]:
                sched.append((tg, 1))
    sched.append((tt - 2, 0))
    sched.append((tt - 1, 1))

    calls = []                       # (chunk, tile_off, n_tiles)
    for c in range(nchunks):
        t = 0
        while t < b_tiles[c]:
            k = min(CALL_TILES, b_tiles[c] - t)
            calls.append((c, int(offs[c]) + t, k))
            t += k

    es = et * 128
    order = np.lexsort((dstloc, chunk_of, sup_of, core))
    so, do, co, ko, ro = (src[order], dstloc[order] % SUP, chunk_of[order],
                          core[order], r_src[order])
    su = sup_of[order]
    normv = (dinv[so] * dinv[dst[order]]).astype(np.float32)

    gkey = (ko * nsup + su) * nchunks + co
    gstart = np.zeros(nk + 1, np.int64)
    np.add.at(gstart, gkey + 1, 1)
    gstart = np.cumsum(gstart)
    within = np.arange(len(so)) - gstart[gkey]
    slot = offs[co] * 128 + within
    p_ = slot % 128
    t_ = slot // 128

    idx_flat = np.zeros((NC, nsup, max(es, 16)), np.int16)
    idx_flat[ko, su, slot] = (ro % CHUNK).astype(np.int16)
    IDX = np.tile(
        idx_flat[:, :, :es].reshape(NC, nsup, es // 16, 16).transpose(0, 1, 3, 2),
        (1, 1, 8, 1)).copy()
    SN = np.zeros((NC, nsup, 128, tt, 2), np.float32)
    SN[..., 0] = 999.0
    SN[ko, su, p_, t_, 0] = do.astype(np.float32)
    SN[ko, su, p_, t_, 1] = normv
    BL = np.full((NC, nsup, 128, 2), 999.0, np.float32)

    ar = np.arange(128)
    for k in range(NC):
        for s in range(nsup):
            g0 = k * nsh + s * SUP
            for half in range(2):
                rows = g0 + 128 * half + ar
                valid = rows < (k + 1) * nsh
                rc = np.minimum(rows, n - 1)
                dv = np.where(valid, dinv[rc], 0.0)
                SN[k, s, :, tt - 2 + half, 0] = 128 * half + ar
                SN[k, s, :, tt - 2 + half, 1] = (dv * dv).astype(np.float32)
                BL[k, s, :, half] = np.where(valid, batch[rc], 999).astype(np.float32)

    cnts = np.bincount(batch, minlength=g).astype(np.float32)
    rcnt = (1.0 / np.maximum(cnts, 1.0)).astype(np.float32).reshape(g, 1)

    return dict(n=n, g=g, nsh=nsh, nsup=nsup, nshp=nshp, npad=npad,
                nchunks=nchunks, b_tiles=tuple(b_tiles), et=et, tt=tt,
                sched=tuple(sched), calls=tuple(calls), es=es,
                IDX=IDX, SN=SN, BL=BL, rcnt=rcnt)


# ---------------------------------------------------------------- program
def _build(pk, phases=(1, 2, 3, 4)):
    import concourse.bacc as bacc
    import concourse.mybir as mybir
    import concourse.tile as tile
    from concourse.library_config import mlp as mlp_lib

    f32 = mybir.dt.float32
    dt2 = mybir.dt.bfloat16 if BF16_T2 else f32
    nsup, tt, npad, nshp = pk["nsup"], pk["tt"], pk["npad"], pk["nshp"]
    es, g = pk["es"], pk["g"]
    sched = pk["sched"]

    nc = bacc.Bacc("TRN2", target_bir_lowering=False, debug=False,
                   num_devices=NC, num_swdge_queues=NQ)

    xpad = nc.dram_tensor("xpad", [npad, F], f32, kind="ExternalInput")
    xown = nc.dram_tensor("xown", [nshp, F], f32, kind="ExternalInput")
    IDXd = nc.dram_tensor("IDX", [nsup, 128, es // 16], mybir.dt.int16,
                          kind="ExternalInput")
    SNd = nc.dram_tensor("SN", [nsup, 128, tt * 2], f32, kind="ExternalInput")
    BLd = nc.dram_tensor("BL", [nsup, 128, 2], f32, kind="ExternalInput")
    Wd = {}
    for nm, shp in [("aW1", [F, H]), ("cW1", [F, H]), ("aW2", [H, H]),
                    ("cW2", [H, H]), ("mW", [H, A]), ("f1W", [H, 64]),
                    ("f2W", [64, 1]), ("ab1", [H, 1]), ("cb1", [H, 1]),
                    ("ab2", [H, 1]), ("cb2", [H, 1]), ("mb", [A, 1]),
                    ("f1b", [64, 1]), ("f2b", [1, 1]), ("logstd", [1, A]),
                    ("rcnt", [g, 1]), ("iota_lo", [128, 128]),
                    ("iota_hi", [128, 128]), ("giota", [128, g]),
                    ("ident", [128, 128])]:
        Wd[nm] = nc.dram_tensor(nm, shp, f32, kind="ExternalInput")

    mean_out = nc.dram_tensor("mean_out", [nshp, A], f32, kind="ExternalOutput")
    value_out = nc.dram_tensor("value_out", [1, g], f32, kind="ExternalOutput")
    std_out = nc.dram_tensor("std_out", [1, A], f32, kind="ExternalOutput")

    w_first = {w: min(i for i, (_, ww) in enumerate(sched) if ww == w) for w in (0, 1)}
    w_last = {w: max(i for i, (_, ww) in enumerate(sched) if ww == w) for w in (0, 1)}

    eq, mu = mybir.AluOpType.is_equal, mybir.AluOpType.mult
    RELU = mybir.ActivationFunctionType.Relu
    TANH = mybir.ActivationFunctionType.Tanh
    EXP = mybir.ActivationFunctionType.Exp

    with tile.TileContext(nc) as tc:
        nc.gpsimd.load_library(mlp_lib)
        ctx = ExitStack()
        cpool = ctx.enter_context(tc.tile_pool(name="consts", bufs=1))
        dram = ctx.enter_context(tc.tile_pool(name="dram", bufs=1, space="DRAM"))

        C = {}
        for nm in Wd:
            t = cpool.tile(list(Wd[nm].shape), f32, name=f"c_{nm}", tag=f"c_{nm}")
            nc.sync.dma_start(t[:], Wd[nm][:])
            C[nm] = t
        iw = [C["iota_lo"], C["iota_hi"]]

        T2own = dram.tile([nshp, 2 * H], dt2, name="T2own", tag="T2own")
        T2full = dram.tile([npad, 2 * H], dt2, name="T2full", tag="T2full",
                           addr_space="Shared")
        ARin = dram.tile([g, H], f32, name="ARin", tag="ARin")
        ARout = dram.tile([g, H], f32, name="ARout", tag="ARout",
                          addr_space="Shared")

        gq = [0]

        def propagate(sup, table, self_table, elem, gbuf_pool, spool, idxp,
                      snp, psum_segs, dt):
            idxt = idxp.tile([128, es // 16], mybir.dt.int16, name="idxt", tag="idxt")
            nc.sync.dma_start(idxt[:], IDXd[sup, :, :])
            snt = snp.tile([128, tt * 2], f32, name="snt", tag="snt")
            nc.sync.dma_start(snt[:], SNd[sup, :, :])
            gb = gbuf_pool.tile([128, tt, elem], dt, name="gb", tag="gb")
            for (c, t0, ntl) in pk["calls"]:
                nidx = ntl * 128
                lo = c * CHUNK
                hi = min(lo + CHUNK, npad)
                nc.gpsimd.dma_gather(
                    gb[:, t0:t0 + ntl, :], table[lo:hi, :],
                    idxt[:, t0 * 8:t0 * 8 + nidx // 16],
                    nidx, nidx, elem, queue_num=gq[0] % NQ)
                gq[0] += 1
            r0 = sup * SUP
            nc.sync.dma_start(
                gb[:, tt - 2:tt, :],
                self_table[r0:r0 + SUP, :].rearrange("(j p) e -> p j e", p=128))
            nmm = elem // 128
            pss = [[psum_segs.tile([128, 128], f32, name=f"ps{m}{w}",
                                   tag=f"ps{m}{w}")
                    for w in (0, 1)] for m in range(nmm)]
            for i, (t, w) in enumerate(sched):
                S = spool.tile([128, 128], dt, name="S", tag="S")
                nc.vector.tensor_scalar(S[:], iw[w][:], snt[:, 2 * t:2 * t + 1],
                                        snt[:, 2 * t + 1:2 * t + 2], eq, mu)
                st, sp = i == w_first[w], i == w_last[w]
                for m in range(nmm):
                    nc.tensor.matmul(pss[m][w][:], lhsT=gb[:, t, 128 * m:128 * (m + 1)],
                                     rhs=S[:], start=st, stop=sp)
            return pss

        # ------- phase 1: propagate x; layer-1 MLPs; build T2own
        if 1 in phases:
          with tc.tile_pool(name="g1", bufs=2) as gp1, \
                tc.tile_pool(name="s1", bufs=6) as sp1, \
                tc.tile_pool(name="ix1", bufs=2) as ixp, \
                tc.tile_pool(name="sn1", bufs=2) as snp, \
                tc.tile_pool(name="pseg1", bufs=2, space="PSUM") as psg, \
                tc.tile_pool(name="pwork1", bufs=3, space="PSUM") as pwk, \
                tc.tile_pool(name="e1", bufs=3) as ep:
            for sup in range(nsup):
                pss = propagate(sup, xpad, xown, F, gp1, sp1, ixp, snp, psg, f32)
                for w in (0, 1):
                    P = ep.tile([128, 128], f32, name="P", tag="P")
                    nc.vector.tensor_copy(P[:], pss[0][w][:])
                    rows0 = sup * SUP + w * 128
                    for Wn, bn, col in (("aW1", "ab1", 0), ("cW1", "cb1", H)):
                        z = pwk.tile([128, 128], f32, name="z", tag="wk")
                        nc.tensor.matmul(z[:], lhsT=C[Wn][:], rhs=P[:],
                                         start=True, stop=True)
                        act = ep.tile([128, 128], f32, name="act", tag="act")
                        nc.scalar.activation(act[:], z[:], RELU, bias=C[bn][:])
                        zt = pwk.tile([128, 128], f32, name="zt", tag="wk")
                        nc.tensor.transpose(zt[:], act[:], C["ident"][:])
                        nm_ = ep.tile([128, 128], dt2, name="nm", tag="nm")
                        nc.vector.tensor_copy(nm_[:], zt[:])
                        nc.sync.dma_start(T2own[rows0:rows0 + 128, col:col + H],
                                          nm_[:])

        # ------- phase 2: exchange layer-2 table
        if 2 in phases:
          nc.gpsimd.collective_compute(
            "AllGather", mybir.AluOpType.bypass,
            replica_groups=[list(range(NC))],
            ins=[T2own.opt()], outs=[T2full.opt()])

        # ------- phase 3: propagate [a1|c1]; heads; pooled partial sums
        if 3 in phases:
          with tc.tile_pool(name="g2", bufs=2) as gp2, \
                tc.tile_pool(name="s2", bufs=6) as sp2, \
                tc.tile_pool(name="ix2", bufs=2) as ixp2, \
                tc.tile_pool(name="sn2", bufs=2) as snp2, \
                tc.tile_pool(name="blp", bufs=2) as blp, \
                tc.tile_pool(name="pseg2", bufs=1, space="PSUM") as psg2, \
                tc.tile_pool(name="ppool", bufs=1, space="PSUM") as ppl, \
                tc.tile_pool(name="pwork2", bufs=3, space="PSUM") as pwk2, \
                tc.tile_pool(name="e2", bufs=3) as ep2:
            pool_ps = ppl.tile([g, H], f32, name="poolps", tag="poolps")
            for sup in range(nsup):
                blt = blp.tile([128, 2], f32, name="blt", tag="blt")
                nc.sync.dma_start(blt[:], BLd[sup, :, :])
                pss = propagate(sup, T2full, T2own, 2 * H, gp2, sp2, ixp2,
                                snp2, psg2, dt2)
                for w in (0, 1):
                    P2a = ep2.tile([128, 128], f32, name="P2a", tag="P2a")
                    nc.vector.tensor_copy(P2a[:], pss[0][w][:])
                    P2c = ep2.tile([128, 128], f32, name="P2c", tag="P2c")
                    nc.vector.tensor_copy(P2c[:], pss[1][w][:])
                    rows0 = sup * SUP + w * 128
                    z = pwk2.tile([128, 128], f32, name="z2", tag="wk2")
                    nc.tensor.matmul(z[:], lhsT=C["aW2"][:], rhs=P2a[:],
                                     start=True, stop=True)
                    a2 = ep2.tile([128, 128], f32, name="a2", tag="a2")
                    nc.scalar.activation(a2[:], z[:], RELU, bias=C["ab2"][:])
                    zm = pwk2.tile([A, 128], f32, name="zm", tag="wk2")
                    nc.tensor.matmul(zm[:], lhsT=C["mW"][:], rhs=a2[:],
                                     start=True, stop=True)
                    mt = ep2.tile([A, 128], f32, name="mt", tag="mt")
                    nc.scalar.activation(mt[:], zm[:], TANH, bias=C["mb"][:])
                    mtp = pwk2.tile([128, A], f32, name="mtp", tag="wk2")
                    nc.tensor.transpose(mtp[:], mt[:], C["ident"][:A, :A])
                    mrow = ep2.tile([128, A], f32, name="mrow", tag="mrow")
                    nc.vector.tensor_copy(mrow[:], mtp[:])
                    nc.sync.dma_start(mean_out[rows0:rows0 + 128, :], mrow[:])
                    zc = pwk2.tile([128, 128], f32, name="zc2", tag="wk2")
                    nc.tensor.matmul(zc[:], lhsT=C["cW2"][:], rhs=P2c[:],
                                     start=True, stop=True)
                    c2 = ep2.tile([128, 128], f32, name="c2", tag="c2")
                    nc.scalar.activation(c2[:], zc[:], RELU, bias=C["cb2"][:])
                    c2tp = pwk2.tile([128, 128], f32, name="c2tp", tag="wk2")
                    nc.tensor.transpose(c2tp[:], c2[:], C["ident"][:])
                    c2n = ep2.tile([128, 128], f32, name="c2n", tag="c2n")
                    nc.vector.tensor_copy(c2n[:], c2tp[:])
                    Sg = ep2.tile([128, g], f32, name="Sg", tag="Sg")
                    nc.vector.tensor_scalar(Sg[:], C["giota"][:], blt[:, w:w + 1],
                                            None, eq)
                    nc.tensor.matmul(pool_ps[:], lhsT=Sg[:], rhs=c2n[:],
                                     start=(sup == 0 and w == 0),
                                     stop=(sup == nsup - 1 and w == 1))
            pool_sb = ep2.tile([g, H], f32, name="pool_sb", tag="pool_sb")
            nc.vector.tensor_copy(pool_sb[:], pool_ps[:])
            nc.sync.dma_start(ARin[:, :], pool_sb[:])

        # ------- phase 4: AllReduce pooled sums; value head; std
        if 4 in phases:
          nc.gpsimd.collective_compute(
            "AllReduce", mybir.AluOpType.add,
            replica_groups=[list(range(NC))],
            ins=[ARin.opt()], outs=[ARout.opt()])
          with tc.tile_pool(name="v", bufs=1) as vp, \
                tc.tile_pool(name="pv", bufs=1, space="PSUM") as pv:
            gx = vp.tile([g, H], f32, name="gx", tag="gx")
            nc.sync.dma_start(gx[:], ARout[:, :])
            nc.vector.tensor_scalar(gx[:], gx[:], C["rcnt"][:], None, mu)
            gxt_p = pv.tile([H, g], f32, name="gxt_p", tag="gxt_p")
            nc.tensor.transpose(gxt_p[:], gx[:], C["ident"][:g, :g])
            gxt = vp.tile([H, g], f32, name="gxt", tag="gxt")
            nc.vector.tensor_copy(gxt[:], gxt_p[:])
            z1 = pv.tile([64, g], f32, name="z1", tag="z1")
            nc.tensor.matmul(z1[:], lhsT=C["f1W"][:], rhs=gxt[:],
                             start=True, stop=True)
            v1 = vp.tile([64, g], f32, name="v1", tag="v1")
            nc.scalar.activation(v1[:], z1[:], RELU, bias=C["f1b"][:])
            zv = pv.tile([1, g], f32, name="zv", tag="zv")
            nc.tensor.matmul(zv[:], lhsT=C["f2W"][:], rhs=v1[:],
                             start=True, stop=True)
            vsb = vp.tile([1, g], f32, name="vsb", tag="vsb")
            nc.vector.tensor_scalar(vsb[:], zv[:], C["f2b"][:], None,
                                    mybir.AluOpType.add)
            nc.sync.dma_start(value_out[:, :], vsb[:])
            es_ = vp.tile([1, A], f32, name="es_", tag="es_")
            nc.scalar.activation(es_[:], C["logstd"][:], EXP)
            nc.sync.dma_start(std_out[:, :], es_[:])
        ctx.close()
    nc.compile()
    return nc


# ---------------------------------------------------------------- runner
def _run(nc, pk, inputs):
    from concourse.bass_utils import run_bass_kernel_spmd

    n, g = pk["n"], pk["g"]
    nsh, nshp, npad, nsup, tt = (pk["nsh"], pk["nshp"], pk["npad"],
                                 pk["nsup"], pk["tt"])

    x = np.ascontiguousarray(np.asarray(inputs["x"], np.float32))
    xpad = np.zeros((npad, F), np.float32)
    for k in range(NC):
        xpad[k * nshp:k * nshp + nsh] = x[k * nsh:(k + 1) * nsh]

    iota = np.tile(np.arange(128, dtype=np.float32), (128, 1))
    common = {
        "xpad": xpad,
        "aW1": np.asarray(inputs["aW1"], np.float32),
        "cW1": np.asarray(inputs["cW1"], np.float32),
        "aW2": np.asarray(inputs["aW2"], np.float32),
        "cW2": np.asarray(inputs["cW2"], np.float32),
        "mW": np.asarray(inputs["mW"], np.float32),
        "f1W": np.asarray(inputs["f1W"], np.float32),
        "f2W": np.asarray(inputs["f2W"], np.float32),
        "ab1": np.asarray(inputs["ab1"], np.float32).reshape(H, 1),
        "cb1": np.asarray(inputs["cb1"], np.float32).reshape(H, 1),
        "ab2": np.asarray(inputs["ab2"], np.float32).reshape(H, 1),
        "cb2": np.asarray(inputs["cb2"], np.float32).reshape(H, 1),
        "mb": np.asarray(inputs["mb"], np.float32).reshape(A, 1),
        "f1b": np.asarray(inputs["f1b"], np.float32).reshape(64, 1),
        "f2b": np.asarray(inputs["f2b"], np.float32).reshape(1, 1),
        "logstd": np.asarray(inputs["log_std"], np.float32).reshape(1, A),
        "rcnt": pk["rcnt"],
        "iota_lo": iota,
        "iota_hi": iota + np.float32(128.0),
        "giota": np.tile(np.arange(g, dtype=np.float32), (128, 1)),
        "ident": np.eye(128, dtype=np.float32),
    }
    in_maps = []
    for k in range(NC):
        m = dict(common)
        m["xown"] = np.ascontiguousarray(xpad[k * nshp:(k + 1) * nshp])
        m["IDX"] = pk["IDX"][k]
        m["SN"] = np.ascontiguousarray(pk["SN"][k].reshape(nsup, 128, tt * 2))
        m["BL"] = pk["BL"][k]
        in_maps.append(m)

    res = run_bass_kernel_spmd(nc, in_maps, core_ids=list(range(NC)))
    _last.update(nc=nc, in_maps=in_maps, pk=pk)
    mean = np.concatenate([res.results[k]["mean_out"][:nsh] for k in range(NC)])
    value = res.results[0]["value_out"].reshape(g, 1)
    std = np.broadcast_to(res.results[0]["std_out"].reshape(1, A), (n, A)).copy()
    return mean, std, value


def kernel(**inputs):
    n = int(np.asarray(inputs["x"]).shape[0])
    g = G
    pk = _pack(np.asarray(inputs["edge_index"]), inputs["batch"], n, g)
    key = (n, g, pk["b_tiles"], pk["sched"])
    if key not in _cache:
        _cache[key] = _build(pk)
    return _run(_cache[key], pk, inputs)


# ------------------------------------------------- timing (test-only helper)
_last = {}


def _make_runner(nc, in_maps):
    """Jitted shard_map runner with device-resident inputs (axon path)."""
    import jax
    from jax.experimental.shard_map import shard_map
    from jax.sharding import Mesh, PartitionSpec

    import concourse.mybir as mybir
    from concourse.bass2jax import (_bass_exec_p, install_neuronx_cc_hook,
                                    partition_id_tensor)

    install_neuronx_cc_hook()
    in_names, out_names, out_avals, zero_outs = [], [], [], []
    pname = nc.partition_id_tensor.name if nc.partition_id_tensor else None
    for alloc in nc.m.functions[0].allocations:
        if not isinstance(alloc, mybir.MemoryLocationSet):
            continue
        name = alloc.memorylocations[0].name
        if alloc.kind == "ExternalInput":
            if name != pname:
                in_names.append(name)
        elif alloc.kind == "ExternalOutput":
            shape = tuple(alloc.tensor_shape)
            dtype = mybir.dt.np(alloc.dtype)
            out_names.append(name)
            out_avals.append(jax.core.ShapedArray(shape, dtype))
            zero_outs.append(np.zeros(shape, dtype))
    all_in = in_names + out_names + ([pname] if pname else [])

    def _body(*args):
        operands = list(args)
        if pname:
            operands.append(partition_id_tensor())
        return tuple(_bass_exec_p.bind(
            *operands, out_avals=tuple(out_avals), in_names=tuple(all_in),
            out_names=tuple(out_names), lowering_input_output_aliases=(),
            sim_require_finite=True, sim_require_nnan=True, nc=nc))

    ncor = len(in_maps)
    mesh = Mesh(np.asarray(jax.devices()[:ncor]), ("core",))
    specs_in = (PartitionSpec("core"),) * (len(in_names) + len(out_names))
    jf = jax.jit(shard_map(_body, mesh=mesh, in_specs=specs_in,
                           out_specs=(PartitionSpec("core"),) * len(out_names),
                           check_rep=False), keep_unused=True)
    dev_in = [jax.device_put(np.concatenate(
        [np.asarray(in_maps[c][nm]) for c in range(ncor)], axis=0))
        for nm in in_names]
    dev_zero = [jax.device_put(np.zeros((ncor * z.shape[0], *z.shape[1:]),
                                        z.dtype)) for z in zero_outs]

    def run():
        outs = jf(*dev_in, *dev_zero)
        jax.block_until_ready(outs)
        return outs

    return run


def _build_null(pk):
    """Same external I/O as the real program, near-empty body."""
    import concourse.bacc as bacc
    import concourse.mybir as mybir
    import concourse.tile as tile

    f32 = mybir.dt.float32
    nsup, tt, npad, nshp, es, g = (pk["nsup"], pk["tt"], pk["npad"],
                                   pk["nshp"], pk["es"], pk["g"])
    nc = bacc.Bacc("TRN2", target_bir_lowering=False, debug=False,
                   num_devices=NC, num_swdge_queues=NQ)
    nc.dram_tensor("xpad", [npad, F], f32, kind="ExternalInput")
    nc.dram_tensor("xown", [nshp, F], f32, kind="ExternalInput")
    nc.dram_tensor("IDX", [nsup, 128, es // 16], mybir.dt.int16, kind="ExternalInput")
    nc.dram_tensor("SN", [nsup, 128, tt * 2], f32, kind="ExternalInput")
    nc.dram_tensor("BL", [nsup, 128, 2], f32, kind="ExternalInput")
    names = [("aW1", [F, H]), ("cW1", [F, H]), ("aW2", [H, H]), ("cW2", [H, H]),
             ("mW", [H, A]), ("f1W", [H, 64]), ("f2W", [64, 1]), ("ab1", [H, 1]),
             ("cb1", [H, 1]), ("ab2", [H, 1]), ("cb2", [H, 1]), ("mb", [A, 1]),
             ("f1b", [64, 1]), ("f2b", [1, 1]), ("logstd", [1, A]),
             ("rcnt", [g, 1]), ("iota_lo", [128, 128]), ("iota_hi", [128, 128]),
             ("giota", [128, g]), ("ident", [128, 128])]
    ten = {nm: nc.dram_tensor(nm, shp, f32, kind="ExternalInput")
           for nm, shp in names}
    mo = nc.dram_tensor("mean_out", [nshp, A], f32, kind="ExternalOutput")
    vo = nc.dram_tensor("value_out", [1, g], f32, kind="ExternalOutput")
    so = nc.dram_tensor("std_out", [1, A], f32, kind="ExternalOutput")
    with tile.TileContext(nc) as tc:
        with tc.tile_pool(name="p", bufs=1) as pool:
            t = pool.tile([1, A], f32, name="t", tag="t")
            nc.sync.dma_start(t[:], ten["logstd"][:])
            nc.sync.dma_start(so[:, :], t[:])
            t2 = pool.tile([1, g], f32, name="t2", tag="t2")
            nc.sync.dma_start(t2[:], ten["rcnt"][:].rearrange("a b -> b a"))
            nc.sync.dma_start(vo[:, :], t2[:])
            t3 = pool.tile([128, A], f32, name="t3", tag="t3")
            nc.sync.dma_start(t3[:], ten["giota"][:, :A])
            nc.sync.dma_start(mo[:128, :], t3[:])
    nc.compile()
    return nc


def measure_exec_ns(pairs=24):
    """Median of interleaved (full - null) wall-time differences."""
    import time
    if "nc" not in _last:
        return None
    run_full = _make_runner(_last["nc"], _last["in_maps"])
    run_null = _make_runner(_build_null(_last["pk"]), _last["in_maps"])

    def t1(run):
        t0 = time.perf_counter()
        run()
        return time.perf_counter() - t0

    for _ in range(3):
        run_full()
        run_null()
    diffs = []
    fs, ns_ = [], []
    for _ in range(pairs):
        a = t1(run_full)
        b = t1(run_null)
        fs.append(a)
        ns_.append(b)
        diffs.append(a - b)
    diffs.sort()
    med = diffs[len(diffs) // 2]
    print(f"  full_min={min(fs) * 1e3:.1f}ms null_min={min(ns_) * 1e3:.1f}ms "
          f"med_diff={med * 1e3:.2f}ms min_diff={(min(fs) - min(ns_)) * 1e3:.2f}ms")
    return int(max(med, min(fs) - min(ns_)) * 1e9)
